# revision 46
# baseline (speedup 1.0000x reference)
"""GatedAttention Trainium2 kernel.

Math (per batch b):
  Qw = x @ Wq + bq            (N, A)
  Kw = x @ Wk + bk            (N, A)
  g  = sigmoid(Qw @ Wv + bv)  (N,)
  S  = Qw @ Kw^T, diag -> -inf
  P  = softmax(S, axis=0)     (column softmax)
  out = (1-g)[:,None] * P + g[:,None] * I

Sharding: 8 cores = 4 batches x 2 column-halves of the score matrix.
Column softmax is independent per column, so no cross-core reduction for
the softmax itself.

Qw dedup: a column-shard core needs Qw for ALL 4096 rows, but each core
only PROJECTS its own 2048 rows (the ones whose x it loads); the other
half arrives from the pair core (same batch, other column half) via a
pair-wise AllGather (DRAM bounce; runs on the TOPSP/SDMA collective
silicon and overlaps compute). The AG output is rank-ordered, so each
core reads the peer slice back with an indirect (row-index) DMA whose
index vector is a per-core host input - the program stays pure SPMD.
The exchange also carries the peer's gate rows, so gate z-matmuls are
local-only. This cuts projection PE work by a third (944 -> 812
matmuls at ~227ns each).

Device layout: scores computed transposed, sT[j, i] tiles (j on
partitions) so the softmax reduction over i is a free-axis reduction.
The i axis is host-permuted so each core's diagonal block sits at
i in [0, 2048).

Dtypes: x / Wq / Wk ship as fp16 (half the HBM read traffic) and are
upcast on-device to fp32r, which streams through the PE at 227ns per
512-row matmul - measurably faster than fp16/bf16 operands (259ns).
SWDGE casting DMAs are ~5x slower than plain ones, so all casts are
explicit DVE ops. Projections/scores accumulate in fp32 PSUM; Exp
output in bf16; the exchange wire format is fp16.

Schedule: Q-projection pass over the 4 local i-blocks first (x fp16
tiles stay staged in SBUF and are re-cast for the K pass), so the two
1MB AllGathers (qwt cols [0:1024] / [1024:2048] + gate rows) fire at
~35us and ~47us and land before the score loop needs the remote half.
The score loop runs a PREFIX of local-column-only tiles as a latency
buffer, then processes tiles to completion; the DVE finalize stream
(~5.6us/tile: reciprocal-scale x (1-g) stt + diag) stays hidden under
the PE-paced score matmuls.
"""
import numpy as np

import concourse.bacc as bacc
import concourse.bass as bass
import concourse.mybir as mybir
import concourse.tile as tile
from concourse.bass_utils import run_bass_kernel_spmd

FP32 = mybir.dt.float32
FP32R = mybir.dt.float32r
FP16 = mybir.dt.float16
BF16 = mybir.dt.bfloat16
I32 = mybir.dt.int32
AF = mybir.ActivationFunctionType
ALU = mybir.AluOpType
AX = mybir.AxisListType

B, N, H, A = 4, 4096, 1024, 512
NSH = N // 2          # per-core column shard / locally projected rows
NEG = -1.0e30
AGR2 = 514            # AG2 bounce rows: 512 qwt + 2 g1 gate rows
PREFIX = 8            # score tiles that run local-cols-only up front

_CACHE = {}


def _build():
    nc = bacc.Bacc("TRN2", target_bir_lowering=False, debug=False, num_devices=8)
    xq = nc.dram_tensor("xq", [H, NSH], FP16, kind="ExternalInput").ap()
    wq = nc.dram_tensor("wq", [H, A], FP16, kind="ExternalInput").ap()
    wk = nc.dram_tensor("wk", [H, A], FP16, kind="ExternalInput").ap()
    misc = nc.dram_tensor("misc", [128, 18], FP32, kind="ExternalInput").ap()
    eye = nc.dram_tensor("eye", [128, 128], FP32, kind="ExternalInput").ap()
    idx = nc.dram_tensor("idx", [128, 10], I32, kind="ExternalInput").ap()
    out = nc.dram_tensor("out", [NSH, N], BF16, kind="ExternalOutput").ap()

    with tile.TileContext(nc) as tc:
        with (
            tc.tile_pool(name="const", bufs=1) as cpool,
            tc.tile_pool(name="proj_out", bufs=1) as qkpool,
            tc.tile_pool(name="bcast", bufs=1) as bcp,
            tc.tile_pool(name="gaterow", bufs=4) as gtmp,
            tc.tile_pool(name="xchg", bufs=4) as xch,
            tc.tile_pool(name="grpool", bufs=2) as grp,
            tc.tile_pool(name="preps", bufs=1, space="PSUM") as preps,
            tc.tile_pool(name="dram", bufs=1, space="DRAM") as dram,
        ):
            # ---- memset-only constants first: the warm-up burst depends
            # only on these, so the PE starts right after the preamble.
            ones_f = cpool.tile([1, 128], FP32, tag="onesf", name="onesf")
            nc.vector.memset(ones_f[:], 1.0)
            ones_r = cpool.tile([1, 128], FP32R, tag="ones", name="ones")
            nc.vector.tensor_copy(ones_r[:], ones_f[:])
            ones_h = cpool.tile([1, 128], FP16, tag="onesh", name="onesh")
            nc.vector.tensor_copy(ones_h[:], ones_f[:])

            # ---- DMA'd constants
            ident = cpool.tile([128, 128], FP32, tag="ident", name="ident")
            nc.sync.dma_start(ident[:], eye)
            misc_sb = cpool.tile([128, 18], FP32, tag="misc", name="misc")
            nc.gpsimd.dma_start(misc_sb[:], misc)
            idx_sb = cpool.tile([128, 10], I32, tag="idx", name="idx")
            nc.gpsimd.dma_start(idx_sb[:], idx)
            identb = cpool.tile([128, 128], BF16, tag="identb", name="identb")
            nc.vector.tensor_copy(identb[:], ident[:])
            dneg = cpool.tile([128, 128], FP32, tag="dneg", name="dneg")
            nc.vector.tensor_scalar(dneg[:], ident[:], NEG, None, op0=ALU.mult)
            misc_r = cpool.tile([128, 18], FP32R, tag="miscr", name="miscr")
            nc.vector.tensor_copy(misc_r[:], misc_sb[:])

            # ---- persistent projection outputs (fp32r) ----
            qwt = [qkpool.tile([128, N], FP32R, tag=f"qwt{a}", name=f"qwt{a}")
                   for a in range(4)]
            kwt = [qkpool.tile([128, NSH], FP32R, tag=f"kwt{a}", name=f"kwt{a}")
                   for a in range(4)]
            # gate broadcast planes (bf16): g1m[p,i] = 1-g_i (all i),
            # gbc[p,i] = g_i (local i only - the diagonal never lands in the
            # remote half)
            g1m_bc = bcp.tile([128, N], BF16, tag="g1mbc", name="g1mbc")
            g_bc = bcp.tile([128, NSH], BF16, tag="gbc", name="gbc")

            # AllGather bounce buffers (DRAM). agin rows: 0-511 = qwt local
            # (row a), 512 = g1 row, 513 = g2 row; cols = one 1024-wide
            # half of the local i range.
            agin = [dram.tile([512, 1024], FP16, tag="agi0", name="agi0"),
                    dram.tile([AGR2, 1024], FP16, tag="agi1", name="agi1")]
            agout = [dram.tile([1024, 1024], FP16, tag="ago0", name="ago0"),
                     dram.tile([2 * AGR2, 1024], FP16, tag="ago1",
                               name="ago1")]

            # ---- projections + fused gate + exchange ----
            with (
                tc.tile_pool(name="wtiles", bufs=1) as wpool,
                tc.tile_pool(name="wstage", bufs=3) as wst,
                tc.tile_pool(name="xstage", bufs=10) as xst,
                tc.tile_pool(name="xslices", bufs=13) as xpool,
                tc.tile_pool(name="projps", bufs=4, space="PSUM") as ppool,
                tc.tile_pool(name="zrowps", bufs=1, space="PSUM") as zpool,
                tc.tile_pool(name="bcps", bufs=1, space="PSUM") as bps,
            ):
                # PE warm-up: keeps the HAM activity monitor busy during the
                # DMA lead-in so the first real matmuls run at full clock.
                warm = ppool.tile([128, 512], FP32, tag="ps", name="warm")
                for _ in range(32):
                    nc.tensor.matmul(warm[0:64, 0:64], ones_r[:, 0:64],
                                     ones_r[:, 0:64], start=True, stop=True)

                def load_w(dram_t, h, lst, tag):
                    wt = wst.tile([128, A], FP16, tag="wst", name="wst")
                    nc.sync.dma_start(wt[:], dram_t[h * 128:(h + 1) * 128, :])
                    wr = wpool.tile([128, A], FP32R, tag=f"{tag}{h}",
                                    name=f"{tag}{h}")
                    nc.vector.tensor_copy(wr[:], wt[:])
                    lst.append(wr)

                wqr, wkr = [], []
                for h in range(8):
                    load_w(wq, h, wqr, "wqr")

                def load_x(ib):
                    # plain fp16 DMAs on sync (SWDGE casting DMAs are slow),
                    # explicit DVE upcasts to fp32r
                    xs = []
                    for h in range(8):
                        xt = xst.tile([128, 512], FP16, tag="xst", name="xst")
                        nc.sync.dma_start(
                            xt[:], xq[h * 128:(h + 1) * 128,
                                      ib * 512:(ib + 1) * 512])
                        xr = xpool.tile([128, 512], FP32R, tag="xr", name="xr")
                        nc.vector.tensor_copy(xr[:], xt[:])
                        xs.append(xr)
                    return xs

                grows = [None] * 4  # (g1m_row fp16, g_row fp16) per i-block

                def emit_gate(ib):
                    # z = Qw @ Wv (dup pair cols so both outputs land on
                    # partition 0); 1-g = sigmoid(-z-bv), g = sigmoid(z+bv)
                    pzc = zpool.tile([2, 512], FP32, tag="zr", name="zr")
                    for a in range(4):
                        nc.tensor.matmul(
                            pzc[:], misc_r[:, 8 + 2 * a:10 + 2 * a],
                            qwt[a][:, ib * 512:(ib + 1) * 512],
                            start=(a == 0), stop=(a == 3))
                    g1 = gtmp.tile([1, 512], FP16, tag="g1", name="g1")
                    nc.scalar.activation(g1[:], pzc[0:1, :], AF.Sigmoid,
                                         scale=-1.0, bias=misc_sb[0:1, 17:18])
                    g2 = gtmp.tile([1, 512], FP16, tag="g2", name="g2")
                    nc.scalar.activation(g2[:], pzc[0:1, :], AF.Sigmoid,
                                         bias=misc_sb[0:1, 16:17])
                    grows[ib] = (g1, g2)

                def emit_bcast(ib):
                    # local planes for i-block ib via rank-1 ones matmuls
                    g1, g2 = grows[ib]
                    sl = slice(ib * 512, (ib + 1) * 512)
                    pb = bps.tile([128, 512], FP32, tag="pb", name="pb")
                    nc.tensor.matmul(pb[:], ones_h[:], g1[:],
                                     start=True, stop=True)
                    nc.vector.tensor_copy(g1m_bc[:, sl], pb[:])
                    pb2 = bps.tile([128, 512], FP32, tag="pb", name="pb")
                    nc.tensor.matmul(pb2[:], ones_h[:], g2[:],
                                     start=True, stop=True)
                    nc.scalar.copy(g_bc[:, sl], pb2[:])

                def emit_exchange(half):
                    # ship qwt[:, half*1024:(half+1)*1024]; ALL g1 gate rows
                    # ride AG2 only (AG2's doorbell is gated by AG1
                    # completion anyway, while AG1's must fire the moment
                    # the first two Q blocks finish)
                    sl = slice(half * 1024, (half + 1) * 1024)
                    for a in range(4):
                        xc = xch.tile([128, 1024], FP16, tag="xc", name="xc")
                        nc.vector.tensor_copy(xc[:], qwt[a][:, sl])
                        # bounce writes split across both queues: the
                        # doorbell waits on all four, so halving per-queue
                        # bytes pulls the collective start earlier
                        eng = nc.gpsimd if a % 2 == 0 else nc.sync
                        eng.dma_start(agin[half][a * 128:(a + 1) * 128, :],
                                      xc[:])
                    if half == 1:
                        for ib in range(4):
                            nc.gpsimd.dma_start(
                                agin[1][512 + ib // 2:513 + ib // 2,
                                        (ib % 2) * 512:(ib % 2 + 1) * 512],
                                grows[ib][0][:])
                    nc.gpsimd.collective_compute(
                        "AllGather", ALU.bypass,
                        replica_groups=[[0, 1], [2, 3], [4, 5], [6, 7]],
                        ins=[agin[half][:].opt()], outs=[agout[half][:].opt()],
                    )

                # ---- merged Q+K pass: one x load/upcast per i-block
                # feeds both projections (the earlier split-pass design
                # re-loaded x and its second DVE cast stream entangled with
                # the exchange downcasts, head-blocking the K pipeline).
                # exchange(1) is emitted after the loop, so its qwt casts
                # land after every x upcast in the in-order DVE queue.
                xs_next = load_x(0)
                for ib in range(4):
                    xs = xs_next
                    if ib == 0:
                        xs_next = load_x(1)
                        for h in range(8):
                            load_w(wk, h, wkr, "wkr")
                    else:
                        xs_next = load_x(ib + 1) if ib < 3 else None
                    for a in range(4):
                        pq = ppool.tile([128, 512], FP32, tag="ps", name="ps")
                        for h in range(8):
                            nc.tensor.matmul(pq[:], wqr[h][:, a * 128:(a + 1) * 128],
                                             xs[h][:], start=(h == 0), stop=(h == 7))
                        nc.scalar.activation(qwt[a][:, ib * 512:(ib + 1) * 512],
                                             pq[:], AF.Identity,
                                             bias=misc_sb[:, a:a + 1])
                        pk = ppool.tile([128, 512], FP32, tag="ps", name="ps")
                        for h in range(8):
                            nc.tensor.matmul(pk[:], wkr[h][:, a * 128:(a + 1) * 128],
                                             xs[h][:], start=(h == 0), stop=(h == 7))
                        nc.scalar.activation(kwt[a][:, ib * 512:(ib + 1) * 512],
                                             pk[:], AF.Identity,
                                             bias=misc_sb[:, 4 + a:5 + a])
                    if ib >= 1:
                        emit_gate(ib - 1)
                    if ib >= 2:
                        emit_bcast(ib - 2)
                    if ib == 2:
                        emit_exchange(0)
                emit_gate(3)
                emit_exchange(1)
                emit_bcast(2)
                emit_bcast(3)

            # ---- exchange readback (indirect gathers run on gpsimd and
            # wait on the collective's completion; emitted in program order
            # AFTER the prefix score tiles so the waits never head-block
            # ready PE work)
            def readback(half):
                sl = slice(NSH + half * 1024, NSH + (half + 1) * 1024)
                for a in range(4):
                    gt = xch.tile([128, 1024], FP16, tag="gt", name="gt")
                    nc.gpsimd.indirect_dma_start(
                        out=gt[:], out_offset=None,
                        in_=agout[half][:],
                        in_offset=bass.IndirectOffsetOnAxis(
                            ap=idx_sb[:, 4 * half + a:4 * half + a + 1],
                            axis=0),
                    )
                    nc.vector.tensor_copy(qwt[a][:, sl], gt[:])

            def g1m_gather():
                # remote (1-g) planes for blocks 4-7: every partition
                # gathers the same gate row (const idx cols 8/9), column
                # offset selects the block half, then DVE upcast to bf16
                for blk in range(4):
                    gr = grp.tile([128, 512], FP16, tag="gr", name="gr")
                    nc.gpsimd.indirect_dma_start(
                        out=gr[:], out_offset=None,
                        in_=agout[1][:],
                        in_offset=bass.IndirectOffsetOnAxis(
                            ap=idx_sb[:, 8 + blk // 2:9 + blk // 2], axis=0),
                        element_offset=(blk % 2) * 512,
                    )
                    sl = slice(NSH + blk * 512, NSH + (blk + 1) * 512)
                    nc.vector.tensor_copy(g1m_bc[:, sl], gr[:])

            # ---- score loop over 16 column tiles (output stays transposed)
            with (
                tc.tile_pool(name="explo", bufs=11) as eplo,
                tc.tile_pool(name="exphi", bufs=3) as ephi,
                tc.tile_pool(name="dsum", bufs=20) as dpool,
                tc.tile_pool(name="diag", bufs=2) as dzpool,
                tc.tile_pool(name="scoreps", bufs=3, space="PSUM") as sps,
            ):
                exp_lo = [None] * 16
                dsums = [None] * 16

                def score_half(t, lo, pool=None):
                    # compute the two 1024-chunks of one half of tile t
                    if lo:
                        et = eplo.tile([128, NSH], BF16, tag="el", name="el")
                        exp_lo[t] = et
                        ds = dpool.tile([128, 4], FP32, tag="ds", name="ds")
                        dsums[t] = ds
                        chs = (0, 1)
                    else:
                        et = ephi.tile([128, NSH], BF16, tag="eh", name="eh")
                        ds = dsums[t]
                        chs = (2, 3)
                    dch = (t * 128) // 1024
                    for ch in chs:
                        ps = (pool or sps).tile([128, 1024], FP32,
                                                tag="sc", name="sc")
                        for sub in range(2):
                            o = ch * 1024 + sub * 512
                            for a in range(4):
                                nc.tensor.matmul(ps[:, sub * 512:(sub + 1) * 512],
                                                 kwt[a][:, t * 128:(t + 1) * 128],
                                                 qwt[a][:, o:o + 512],
                                                 start=(a == 0), stop=(a == 3))
                        if ch == dch:
                            off = t * 128 - ch * 1024
                            nc.vector.tensor_add(ps[:, off:off + 128],
                                                 ps[:, off:off + 128], dneg[:])
                        co = (ch - chs[0]) * 1024
                        nc.scalar.activation(et[:, co:co + 1024], ps[:],
                                             AF.Exp, accum_out=ds[:, ch:ch + 1])
                    return et

                def finalize(t, ehi, last=False):
                    elo = exp_lo[t]
                    ds = dsums[t]
                    rcol = dpool.tile([128, 1], FP32, tag="r", name="r")
                    nc.vector.tensor_reduce(rcol[:], ds[:], axis=AX.X,
                                            op=ALU.add)
                    nc.vector.reciprocal(rcol[:], rcol[:])
                    eng, eng2 = ((nc.sync, nc.gpsimd) if t % 2 == 1
                                 else (nc.gpsimd, nc.sync))

                    def do_lo():
                        for ch in range(2):
                            sl = slice(ch * 1024, (ch + 1) * 1024)
                            nc.vector.scalar_tensor_tensor(
                                elo[:, sl], elo[:, sl], rcol[:],
                                g1m_bc[:, sl], op0=ALU.mult, op1=ALU.mult)
                        dz = dzpool.tile([128, 128], BF16, tag="dz", name="dz")
                        nc.vector.tensor_mul(dz[:], identb[:],
                                             g_bc[:, t * 128:(t + 1) * 128])
                        nc.vector.tensor_add(elo[:, t * 128:(t + 1) * 128],
                                             elo[:, t * 128:(t + 1) * 128],
                                             dz[:])
                        eng.dma_start(out[t * 128:(t + 1) * 128, 0:NSH],
                                      elo[:])

                    def do_hi():
                        for ch in range(2):
                            sl = slice(ch * 1024, (ch + 1) * 1024)
                            nc.vector.scalar_tensor_tensor(
                                ehi[:, sl], ehi[:, sl], rcol[:],
                                g1m_bc[:, NSH + ch * 1024:NSH + (ch + 1) * 1024],
                                op0=ALU.mult, op1=ALU.mult)
                            if last:
                                # quarter DMAs, emitted per-stt so the tail
                                # drains through both queues immediately
                                e = eng2 if ch == 0 else eng
                                e.dma_start(
                                    out[t * 128:(t + 1) * 128,
                                        NSH + ch * 1024:NSH + (ch + 1) * 1024],
                                    ehi[:, sl])
                        if not last:
                            eng2.dma_start(out[t * 128:(t + 1) * 128, NSH:N],
                                           ehi[:])

                    if last:
                        # hi exps land last, so its normalize+DMA is the
                        # critical tail: run it first
                        do_hi()
                        do_lo()
                    else:
                        do_lo()
                        do_hi()

                # prefix: local-column halves only. Drawn from the outer
                # 2-bank PSUM pool: the score-scope pools can't allocate
                # until the projection-scope PSUM frees (~92us), which is
                # what actually caused the recurring pre-AG dry window.
                for t in range(PREFIX):
                    score_half(t, lo=True, pool=preps)
                # remote half readback + plane gathers
                readback(0)
                readback(1)
                g1m_gather()
                # drain prefix tiles, then the rest fully
                for t in range(16):
                    if t >= PREFIX:
                        score_half(t, lo=True)
                    ehi = score_half(t, lo=False)
                    finalize(t, ehi, last=(t == 15))
    nc.compile()
    return nc


def kernel(x, Wq, bq, Wk, bk, Wv, bv, _trace=False, _tmpdir=None):
    x = np.asarray(x, dtype=np.float32)
    if "nc" not in _CACHE:
        _CACHE["nc"] = _build()
    nc = _CACHE["nc"]

    bv_f = np.float32(np.asarray(bv).reshape(())[()])
    eye_np = np.eye(128, dtype=np.float32)
    misc = np.zeros((128, 18), dtype=np.float32)
    misc[:, 0:4] = np.asarray(bq, np.float32).reshape(4, 128).T
    misc[:, 4:8] = np.asarray(bk, np.float32).reshape(4, 128).T
    wv_c = np.asarray(Wv, np.float32).reshape(4, 128).T
    misc[:, 8:16:2] = wv_c
    misc[:, 9:16:2] = wv_c
    misc[:, 16] = bv_f
    misc[:, 17] = -bv_f
    wq_np = np.ascontiguousarray(np.asarray(Wq, np.float32).astype(np.float16))
    wk_np = np.ascontiguousarray(np.asarray(Wk, np.float32).astype(np.float16))

    in_maps = []
    for c in range(8):
        b, h = c // 2, c % 2
        # local rows only: perm-i [0,2048) = orig rows [h*2048,(h+1)*2048)
        xqc = np.ascontiguousarray(
            x[b].T[:, h * NSH:(h + 1) * NSH].astype(np.float16))
        idx = np.zeros((128, 10), dtype=np.int32)
        for a in range(4):
            idx[:, a] = (1 - h) * 512 + a * 128 + np.arange(128)
            idx[:, 4 + a] = (1 - h) * AGR2 + a * 128 + np.arange(128)
        idx[:, 8] = (1 - h) * AGR2 + 512   # broadcast rows: all partitions
        idx[:, 9] = (1 - h) * AGR2 + 513   # read the same gate row
        in_maps.append({"xq": xqc, "wq": wq_np, "wk": wk_np, "misc": misc,
                        "eye": eye_np, "idx": idx})

    res = run_bass_kernel_spmd(nc, in_maps, list(range(8)), trace=_trace,
                               tmpdir=_tmpdir)

    outp = np.empty((B, N, N), dtype=np.float32)
    for c in range(8):
        b, h = c // 2, c % 2
        O = np.asarray(res.results[c]["out"]).astype(np.float32).T  # (i_perm, j)
        js = slice(h * NSH, (h + 1) * NSH)
        outp[b, h * NSH:(h + 1) * NSH, js] = O[:NSH]
        outp[b, (1 - h) * NSH:(2 - h) * NSH, js] = O[NSH:]
    if _trace:
        return outp, res
    return outp
